# revision 1
# baseline (speedup 1.0000x reference)
"""GATv2Conv (PyG semantics) on 8 Trainium2 NeuronCores.

Sharding: one attention head per core (H=8 == n_cores). Each core:
  Phase A: x_l_h = x @ W_l[:, h], x_r_h = x @ W_r[:, h] on TensorE (bf16),
           packed as a [N, 128] bf16 table row [x_l | x_r] in HBM.
  Phase B: destination-major bucketed edge phase. Nodes are degree-sorted
           into tiles of 128 (one dst node per partition); each tile has
           S_t slots (max degree in tile). Neighbor source rows are fetched
           with dma_gather (SWDGE indexed gather, 256B rows), attention
           logits + segment softmax + weighted aggregation run on DVE/ACT
           entirely along the free dimension, ELU + residual fused at the
           end. Host does index preprocessing only; all FLOPs on device.

kernel(**inputs) takes the full unsharded inputs and returns the full
[10000, 512] float32 output.
"""

import os
import sys
from contextlib import ExitStack

for _p in ("/opt/trn_rl_repo",):
    if _p not in sys.path:
        sys.path.insert(0, _p)

import numpy as np
import ml_dtypes

N, E, D, H = 10000, 160000, 512, 8
C = D // H  # 64
NT = (N + 127) // 128  # 79 node tiles
NPAD = NT * 128  # 10112
NEG_SLOPE = 0.2
BF16 = ml_dtypes.bfloat16

_CACHE = {}


# ----------------------------------------------------------------------------
# Host-side graph preprocessing (index manipulation only)
# ----------------------------------------------------------------------------

def _wrap_idx(idx_flat):
    """Wrap a logical index list into the SWDGE layout: logical j lives at
    [j % 16, j // 16] of a [16, L] block, replicated across the 8 Q7 core
    stripes -> [128, L] int16."""
    n = len(idx_flat)
    lw = (n + 15) // 16
    buf = np.zeros((16, lw), np.int16)
    j = np.arange(n)
    buf[j % 16, j // 16] = idx_flat.astype(np.int16)
    return np.tile(buf, (8, 1))


def _prep(edge_index):
    ei = np.asarray(edge_index).astype(np.int64)
    src = np.concatenate([ei[0], np.arange(N, dtype=np.int64)])
    dst = np.concatenate([ei[1], np.arange(N, dtype=np.int64)])
    deg = np.bincount(dst, minlength=N)  # >= 1 (self loops)
    order = np.argsort(dst, kind="stable")
    src_sorted = src[order]
    starts = np.zeros(N + 1, np.int64)
    starts[1:] = np.cumsum(deg)
    perm = np.argsort(-deg, kind="stable")  # descending degree
    perm_full = np.concatenate([perm, np.full(NPAD - N, -1, np.int64)])

    S_list, src_blocks, mask_blocks = [], [], []
    for t in range(NT):
        nodes = perm_full[t * 128:(t + 1) * 128]
        degs = np.where(nodes >= 0, deg[np.clip(nodes, 0, N - 1)], 1)
        S = int(degs.max())
        S += S & 1  # even
        S = max(S, 2)
        blk = np.zeros((S, 128), np.int64)
        msk = np.full((128, S), -1e30, np.float32)
        for p in range(128):
            nd = nodes[p]
            if nd < 0:
                msk[p, 0] = 0.0
                continue
            d_ = int(deg[nd])
            blk[:d_, p] = src_sorted[starts[nd]:starts[nd] + d_]
            msk[p, :d_] = 0.0
        S_list.append(S)
        src_blocks.append(blk.reshape(-1))  # logical j = s*128 + p
        mask_blocks.append(msk)

    srcidx = _wrap_idx(np.concatenate(src_blocks))
    nodeidx = _wrap_idx(np.where(perm_full >= 0, perm_full, 0))
    mask = np.concatenate(mask_blocks, axis=1)  # [128, sum(S)] 0 / -1e30
    mask01 = (mask == 0.0).astype(np.float32)   # 1 valid / 0 pad
    perm_clip = np.where(perm_full >= 0, perm_full, 0)
    return dict(S_list=S_list, srcidx=srcidx, nodeidx=nodeidx, mask=mask,
                mask01=mask01, perm=perm, perm_clip=perm_clip)


# ----------------------------------------------------------------------------
# Device program (identical for all 8 cores; per-core data differs)
# ----------------------------------------------------------------------------

def _build(S_list, sumS):
    import concourse.bacc as bacc
    import concourse.tile as tile
    from concourse import mybir

    f32 = mybir.dt.float32
    i16 = mybir.dt.int16
    AF = mybir.ActivationFunctionType
    OP = mybir.AluOpType
    AX = mybir.AxisListType

    LS = 8 * sumS
    nc = bacc.Bacc("TRN2", target_bir_lowering=False, debug=False,
                   num_devices=H)

    xT = nc.dram_tensor("xT", [D, NPAD], f32, kind="ExternalInput")
    Wl = nc.dram_tensor("Wl", [128, 4 * C], f32, kind="ExternalInput")
    Wr = nc.dram_tensor("Wr", [128, 4 * C], f32, kind="ExternalInput")
    attr = nc.dram_tensor("attr", [128, C], f32, kind="ExternalInput")
    biasr = nc.dram_tensor("biasr", [128, C], f32, kind="ExternalInput")
    xres = nc.dram_tensor("xres", [NPAD, C], f32, kind="ExternalInput")
    srci = nc.dram_tensor("srci", [128, LS], i16, kind="ExternalInput")
    nodei = nc.dram_tensor("nodei", [128, 8 * NT], i16, kind="ExternalInput")
    maskd = nc.dram_tensor("maskd", [128, sumS], f32, kind="ExternalInput")
    mask1d = nc.dram_tensor("mask1d", [128, sumS], f32, kind="ExternalInput")
    table_l = nc.dram_tensor("table_l", [NPAD, C], f32)
    table_r = nc.dram_tensor("table_r", [NPAD, C], f32)
    outd = nc.dram_tensor("out", [NPAD, C], f32, kind="ExternalOutput")

    with tile.TileContext(nc) as tc, ExitStack() as ctx:
        res = ctx.enter_context(tc.tile_pool(name="res", bufs=1))
        srci_sb = res.tile([128, LS], i16, tag="srci")
        nc.sync.dma_start(srci_sb[:], srci.ap())
        nodei_sb = res.tile([128, 8 * NT], i16, tag="nodei")
        nc.sync.dma_start(nodei_sb[:], nodei.ap())
        mask_sb = res.tile([128, sumS], f32, tag="mask")
        nc.sync.dma_start(mask_sb[:], maskd.ap())
        mask1_sb = res.tile([128, sumS], f32, tag="mask1")
        nc.sync.dma_start(mask1_sb[:], mask1d.ap())
        att_sb = res.tile([128, C], f32, tag="att")
        nc.sync.dma_start(att_sb[:], attr.ap())
        bias_sb = res.tile([128, C], f32, tag="bias")
        nc.sync.dma_start(bias_sb[:], biasr.ap())

        # ---- Phase A: x @ W_l / x @ W_r (f32) -> f32 tables in HBM ----
        GT = 10  # node tiles per xT streaming group
        with ExitStack() as actx:
            apool = actx.enter_context(tc.tile_pool(name="phA", bufs=2))
            wpool = actx.enter_context(tc.tile_pool(name="phA_w", bufs=1))
            psum = actx.enter_context(
                tc.tile_pool(name="phA_psum", bufs=4, space="PSUM"))
            stg = actx.enter_context(tc.tile_pool(name="phA_stage", bufs=4))
            wl_sb = wpool.tile([128, 4 * C], f32, tag="wl")
            nc.sync.dma_start(wl_sb[:], Wl.ap())
            wr_sb = wpool.tile([128, 4 * C], f32, tag="wr")
            nc.sync.dma_start(wr_sb[:], Wr.ap())
            for g0 in range(0, NT, GT):
                g1 = min(g0 + GT, NT)
                gw = (g1 - g0) * 128
                xts = []
                for k in range(4):
                    xt_k = apool.tile([128, GT * 128], f32, tag=f"xt{k}")
                    nc.sync.dma_start(xt_k[:, 0:gw],
                                      xT[k * 128:(k + 1) * 128,
                                         g0 * 128:g0 * 128 + gw])
                    xts.append(xt_k)
                for t in range(g0, g1):
                    lo = (t - g0) * 128
                    pl = psum.tile([128, C], f32, tag="pl")
                    pr = psum.tile([128, C], f32, tag="pr")
                    for k in range(4):
                        nc.tensor.matmul(pl[:], xts[k][:, lo:lo + 128],
                                         wl_sb[:, k * C:(k + 1) * C],
                                         start=(k == 0), stop=(k == 3))
                    for k in range(4):
                        nc.tensor.matmul(pr[:], xts[k][:, lo:lo + 128],
                                         wr_sb[:, k * C:(k + 1) * C],
                                         start=(k == 0), stop=(k == 3))
                    sl = stg.tile([128, C], f32, tag="sl")
                    nc.scalar.copy(sl[:], pl[:])
                    nc.sync.dma_start(table_l[t * 128:(t + 1) * 128, :], sl[:])
                    sr = stg.tile([128, C], f32, tag="sr")
                    nc.scalar.copy(sr[:], pr[:])
                    nc.sync.dma_start(table_r[t * 128:(t + 1) * 128, :], sr[:])

        # ---- Phase B: edge phase, one dst-node tile per iteration ----
        bp = ctx.enter_context(tc.tile_pool(name="phB", bufs=4))
        sp = ctx.enter_context(tc.tile_pool(name="phB_small", bufs=4))
        pS = 0
        for t, S in enumerate(S_list):
            G = bp.tile([128, S * C], f32, tag="G")
            G3 = G[:].rearrange("p (s c) -> p s c", c=C)
            nc.gpsimd.dma_gather(G3, table_l.ap(),
                                 srci_sb[:, 8 * pS:8 * pS + 8 * S],
                                 128 * S, 128 * S, C,
                                 single_packet=False)
            R = sp.tile([128, C], f32, tag="R")
            nc.gpsimd.dma_gather(R[:].rearrange("p (s c) -> p s c", c=C),
                                 table_r.ap(), nodei_sb[:, 8 * t:8 * t + 8],
                                 128, 128, C)
            xr = sp.tile([128, C], f32, tag="xr")
            nc.sync.dma_start(xr[:], xres[t * 128:(t + 1) * 128, :])

            z = bp.tile([128, S * C], f32, tag="z")
            z3 = z[:].rearrange("p (s c) -> p s c", c=C)
            Rb = R[:].unsqueeze(1).broadcast_to([128, S, C])
            nc.vector.tensor_tensor(z3, G3, Rb, OP.add)
            # lrelu(z) = max(0.2*z, z)
            nc.vector.scalar_tensor_tensor(z[:], z[:], NEG_SLOPE, z[:],
                                           OP.mult, OP.max)
            v = bp.tile([128, S * C], f32, tag="v")
            v3 = v[:].rearrange("p (s c) -> p s c", c=C)
            Ab = att_sb[:].unsqueeze(1).broadcast_to([128, S, C])
            nc.vector.tensor_tensor(v3, z3, Ab, OP.mult)
            alpha = sp.tile([128, S], f32, tag="alpha")
            nc.vector.tensor_reduce(alpha[:], v3, AX.X, OP.add)
            # mask pads to 0 for the segment-sum shift (platform reference
            # computes segment_sum where segment_max was intended)
            nc.vector.tensor_tensor(alpha[:], alpha[:],
                                    mask1_sb[:, pS:pS + S], OP.mult)
            ssumn = sp.tile([128, 1], f32, tag="ssumn")
            nc.vector.tensor_reduce(ssumn[:], alpha[:], AX.X, OP.add,
                                    negate=True)
            # pads to -1e30 for the exp
            nc.vector.tensor_tensor(alpha[:], alpha[:],
                                    mask_sb[:, pS:pS + S], OP.add)
            ea = sp.tile([128, S], f32, tag="ea")
            nc.scalar.activation(ea[:], alpha[:], AF.Exp,
                                 bias=ssumn[:, 0:1], scale=1.0)
            den = sp.tile([128, 1], f32, tag="den")
            nc.vector.tensor_reduce(den[:], ea[:], AX.X, OP.add)
            denc = sp.tile([128, 1], f32, tag="denc")
            nc.vector.tensor_scalar_max(denc[:], den[:], 1e-16)
            rden = sp.tile([128, 1], f32, tag="rden")
            nc.vector.reciprocal(rden[:], denc[:])

            wg = bp.tile([128, S * C], f32, tag="v")
            wg3 = wg[:].rearrange("p (s c) -> p s c", c=C)
            Eb = ea[:].unsqueeze(2).broadcast_to([128, S, C])
            nc.vector.tensor_tensor(wg3, G3, Eb, OP.mult)
            agg = sp.tile([128, C], f32, tag="agg")
            wgT = wg[:].rearrange("p (s c) -> p c s", c=C)
            nc.vector.tensor_reduce(agg[:], wgT, AX.X, OP.add)

            a2 = sp.tile([128, C], f32, tag="a2")
            nc.vector.scalar_tensor_tensor(a2[:], agg[:], rden[:, 0:1],
                                           bias_sb[:], OP.mult, OP.add)
            t1 = sp.tile([128, C], f32, tag="t1")
            nc.vector.tensor_scalar_min(t1[:], a2[:], 0.0)
            u = sp.tile([128, C], f32, tag="u")
            nc.scalar.activation(u[:], t1[:], AF.Exp)
            e1 = sp.tile([128, C], f32, tag="e1")
            nc.vector.scalar_tensor_tensor(e1[:], a2[:], 0.0, u[:],
                                           OP.max, OP.add)
            ot = sp.tile([128, C], f32, tag="ot")
            nc.vector.scalar_tensor_tensor(ot[:], e1[:], -1.0, xr[:],
                                           OP.add, OP.add)
            nc.sync.dma_start(outd[t * 128:(t + 1) * 128, :], ot[:])
            pS += S

    nc.compile()
    return nc


# ----------------------------------------------------------------------------
# Per-core input assembly + driver
# ----------------------------------------------------------------------------

def _make_in_maps(x, W_l, W_r, att, bias, prep):
    xTp = np.zeros((D, NPAD), np.float32)
    xTp[:, :N] = x.T
    xres_all = x[prep["perm_clip"]]  # [NPAD, D] f32

    in_maps = []
    for h in range(H):
        cs = slice(h * C, (h + 1) * C)

        def wchunks(W):
            return np.ascontiguousarray(
                W[:, cs].reshape(4, 128, C).transpose(1, 0, 2).reshape(128, 4 * C)
            ).astype(np.float32)

        in_maps.append({
            "xT": xTp,
            "Wl": wchunks(W_l),
            "Wr": wchunks(W_r),
            "attr": np.ascontiguousarray(np.tile(att[h], (128, 1))).astype(np.float32),
            "biasr": np.ascontiguousarray(
                np.tile(bias[cs], (128, 1))).astype(np.float32),
            "xres": np.ascontiguousarray(xres_all[:, cs]).astype(np.float32),
            "srci": prep["srcidx"],
            "nodei": prep["nodeidx"],
            "maskd": prep["mask"],
            "mask1d": prep["mask01"],
        })
    return in_maps


def _get_program(S_list):
    key = tuple(S_list)
    if key not in _CACHE:
        _CACHE[key] = _build(list(S_list), int(sum(S_list)))
    return _CACHE[key]


_LAST = {}


def kernel(**inputs):
    x = np.asarray(inputs["x"], np.float32)
    edge_index = np.asarray(inputs["edge_index"])
    W_l = np.asarray(inputs["W_l"], np.float32)
    W_r = np.asarray(inputs["W_r"], np.float32)
    att = np.asarray(inputs["att"], np.float32)
    bias = np.asarray(inputs["bias"], np.float32)

    prep = _prep(edge_index)
    nc = _get_program(prep["S_list"])
    in_maps = _make_in_maps(x, W_l, W_r, att, bias, prep)

    from concourse.bass_utils import run_bass_kernel_spmd
    bkr = run_bass_kernel_spmd(nc, in_maps, core_ids=list(range(H)))

    out = np.empty((N, D), np.float32)
    for h in range(H):
        out[prep["perm"], h * C:(h + 1) * C] = bkr.results[h]["out"][:N]

    _LAST["nc"] = nc
    _LAST["in_maps"] = in_maps
    _LAST["prep"] = prep
    return out


def _time_pjrt(nc, in_maps, iters=8):
    """Time the NEFF execution through PJRT with device-resident inputs.
    Returns list of per-call wall times (s)."""
    import time
    import jax
    import numpy as _np
    from jax.sharding import Mesh, PartitionSpec, NamedSharding
    from jax.experimental.shard_map import shard_map
    from concourse import mybir
    from concourse.bass2jax import (_bass_exec_p, install_neuronx_cc_hook,
                                    partition_id_tensor)

    install_neuronx_cc_hook()
    n_cores = len(in_maps)
    partition_name = nc.partition_id_tensor.name if nc.partition_id_tensor else None
    in_names, out_names, out_avals, zero_outs = [], [], [], []
    for alloc in nc.m.functions[0].allocations:
        if not isinstance(alloc, mybir.MemoryLocationSet):
            continue
        name = alloc.memorylocations[0].name
        if alloc.kind == "ExternalInput":
            if name != partition_name:
                in_names.append(name)
        elif alloc.kind == "ExternalOutput":
            out_names.append(name)
            shape = tuple(alloc.tensor_shape)
            dtype = mybir.dt.np(alloc.dtype)
            out_avals.append(jax.core.ShapedArray(shape, dtype))
            zero_outs.append(_np.zeros(shape, dtype))
    n_params = len(in_names)
    full_in_names = in_names + out_names + ([partition_name] if partition_name else [])
    donate = tuple(range(n_params, n_params + len(out_names)))

    def _body(*args):
        operands = list(args)
        if partition_name is not None:
            operands.append(partition_id_tensor())
        return tuple(_bass_exec_p.bind(
            *operands, out_avals=tuple(out_avals), in_names=tuple(full_in_names),
            out_names=tuple(out_names), lowering_input_output_aliases=(),
            sim_require_finite=True, sim_require_nnan=True, nc=nc))

    devices = jax.devices()[:n_cores]
    mesh = Mesh(_np.asarray(devices), ("core",))
    spec = NamedSharding(mesh, PartitionSpec("core"))
    in_specs = (PartitionSpec("core"),) * (n_params + len(out_names))
    out_specs = (PartitionSpec("core"),) * len(out_names)
    fn = jax.jit(shard_map(_body, mesh=mesh, in_specs=in_specs,
                           out_specs=out_specs, check_rep=False),
                 donate_argnums=donate, keep_unused=True)
    concat_in = [jax.device_put(
        _np.concatenate([_np.asarray(in_maps[c][nm]) for c in range(n_cores)], axis=0),
        spec) for nm in in_names]

    def timed_chain(k):
        zero_sets = []
        for _ in range(k):
            zs = [jax.device_put(
                _np.zeros((n_cores * z.shape[0], *z.shape[1:]), z.dtype), spec)
                for z in zero_outs]
            for a in zs:
                a.block_until_ready()
            zero_sets.append(zs)
        t0 = time.perf_counter()
        outs = None
        for zs in zero_sets:
            outs = fn(*concat_in, *zs)
        for o in outs:
            o.block_until_ready()
        return time.perf_counter() - t0

    timed_chain(1)  # warm
    times = {}
    for k in (1, 8):
        times[k] = min(timed_chain(k) for _ in range(max(2, iters // 4)))
    return times


def _null_program():
    import concourse.bacc as bacc
    import concourse.tile as tile
    from concourse import mybir
    nc = bacc.Bacc("TRN2", target_bir_lowering=False, debug=False,
                   num_devices=H)
    a = nc.dram_tensor("a", [128, 64], mybir.dt.float32, kind="ExternalInput")
    o = nc.dram_tensor("out", [128, 64], mybir.dt.float32, kind="ExternalOutput")
    with tile.TileContext(nc) as tc, ExitStack() as ctx:
        p = ctx.enter_context(tc.tile_pool(name="p", bufs=1))
        t = p.tile([128, 64], mybir.dt.float32)
        nc.sync.dma_start(t[:], a.ap())
        nc.sync.dma_start(o.ap(), t[:])
    nc.compile()
    return nc


def profile_exec_ns():
    """Slope-based wall-clock timing through PJRT (no NTFF hook available in
    this container): issue K pipelined executions, marginal cost per call =
    (t_K - t_1) / (K - 1). Returns (exec_time_ns, dict of raw timings)."""
    assert "nc" in _LAST, "call kernel() first"
    tk = _time_pjrt(_LAST["nc"], _LAST["in_maps"])
    ns = int((tk[8] - tk[1]) / 7 * 1e9)
    return ns, {"kernel_chain_s": tk}



# revision 11
# speedup vs baseline: 1.0830x; 1.0830x over previous
"""GATv2Conv (PyG semantics) on 8 Trainium2 NeuronCores — v2.

Sharding: one attention head per core (H=8 == n_cores). Each core:
  Phase A: combined table row [x_l | x_r] (128 bf16 ch = 256B) per node,
           computed as bf16 matmuls on TensorE, staged in SBUF, one DMA out.
  Phase B: destination-major batched edge phase. Degree-sorted dst tiles of
           128 nodes are grouped into batches (B tiles padded to a common S,
           B*S <= CAP slots). Per batch: one 256B-row dma_gather for
           neighbor rows, z-add / att-mult on DVE in bf16 (2x mode), lrelu
           on ACT (Prelu), alpha via bf16 fold-tree (f32 tail), softmax
           shift via negate-max, exp on ACT with broadcast expansion,
           weighted aggregation via bf16 fold-tree. Finale (ELU + residual)
           batched over all tiles at once.

kernel(**inputs) takes full unsharded inputs, returns full [10000, 512] f32.
"""

import os
import sys
from contextlib import ExitStack

for _p in ("/opt/trn_rl_repo",):
    if _p not in sys.path:
        sys.path.insert(0, _p)

import numpy as np
import ml_dtypes

N, E, D, H = 10000, 160000, 512, 8
C = D // H  # 64
NT = (N + 127) // 128  # 79 node tiles
NPAD = NT * 128  # 10112
NEG_SLOPE = 0.2
BF16 = ml_dtypes.bfloat16
CAP = 120  # max slots (B tiles * S slots) per batch (SWDGE ring: <=124)

_CACHE = {}


# ----------------------------------------------------------------------------
# Host-side graph preprocessing (index manipulation only)
# ----------------------------------------------------------------------------

def _wrap_idx(idx_flat):
    """SWDGE index layout: logical j at [j % 16, j // 16] of a [16, L] block,
    replicated across the 8 Q7 core stripes -> [128, L] int16."""
    n = len(idx_flat)
    lw = (n + 15) // 16
    buf = np.zeros((16, lw), np.int16)
    j = np.arange(n)
    buf[j % 16, j // 16] = idx_flat.astype(np.int16)
    return np.tile(buf, (8, 1))


def _prep(edge_index):
    ei = np.asarray(edge_index).astype(np.int64)
    src = np.concatenate([ei[0], np.arange(N, dtype=np.int64)])
    dst = np.concatenate([ei[1], np.arange(N, dtype=np.int64)])
    deg = np.bincount(dst, minlength=N)  # >= 1 (self loops)
    order = np.argsort(dst, kind="stable")
    src_sorted = src[order]
    starts = np.zeros(N + 1, np.int64)
    starts[1:] = np.cumsum(deg)
    perm = np.argsort(-deg, kind="stable")  # descending degree
    perm_full = np.concatenate([perm, np.full(NPAD - N, -1, np.int64)])

    tile_S = []
    for t in range(NT):
        nodes = perm_full[t * 128:(t + 1) * 128]
        degs = np.where(nodes >= 0, deg[np.clip(nodes, 0, N - 1)], 1)
        S = int(degs.max())
        S += S & 1  # even
        S = max(S, 2)
        tile_S.append(S)

    # batches: consecutive tiles share padded S = max S in batch (tiles are
    # degree-sorted so S is non-increasing); B*S <= CAP (B >= 1 always).
    batches = []  # (t0, B, S)
    t0 = 0
    while t0 < NT:
        S = tile_S[t0]
        B = max(1, min(CAP // S, NT - t0))
        batches.append((t0, B, S))
        t0 += B

    src_blocks, mask_blocks = [], []
    for (t0, B, S) in batches:
        blk = np.zeros((B * S, 128), np.int64)  # row j=(b*S+s), col p
        msk = np.full((128, B * S), -1e30, np.float32)
        for b in range(B):
            nodes = perm_full[(t0 + b) * 128:(t0 + b + 1) * 128]
            for p in range(128):
                nd = nodes[p]
                if nd < 0:
                    continue
                d_ = int(deg[nd])
                blk[b * S:b * S + d_, p] = src_sorted[starts[nd]:starts[nd] + d_]
                msk[p, b * S:b * S + d_] = 0.0
        src_blocks.append(blk.reshape(-1))  # logical j = (b*S+s)*128 + p
        mask_blocks.append(msk)

    srcidx = _wrap_idx(np.concatenate(src_blocks))
    nodeidx = _wrap_idx(np.where(perm_full >= 0, perm_full, 0))
    mask = np.concatenate(mask_blocks, axis=1)  # [128, sum(B*S)]
    mask01 = (mask == 0.0).astype(np.float32)   # 1 valid / 0 pad
    perm_clip = np.where(perm_full >= 0, perm_full, 0)
    return dict(batches=batches, srcidx=srcidx, nodeidx=nodeidx, mask=mask,
                mask01=mask01, perm=perm, perm_clip=perm_clip)


# ----------------------------------------------------------------------------
# Device program (identical for all 8 cores; per-core data differs)
# ----------------------------------------------------------------------------

def _fold_widths(S):
    """Fold schedule: w -> ceil(w/2) by adding the top floor(w/2) chunk onto
    the bottom. Returns list of (w, h) with h = w - w//2."""
    out = []
    w = S
    while w > 1:
        h = w - w // 2
        out.append((w, h))
        w = h
    return out


def _build(batches, sumBS):
    import concourse.bacc as bacc
    import concourse.tile as tile
    from concourse import mybir

    f32 = mybir.dt.float32
    bf16 = mybir.dt.bfloat16
    i16 = mybir.dt.int16
    AF = mybir.ActivationFunctionType
    OP = mybir.AluOpType
    AX = mybir.AxisListType

    nc = bacc.Bacc("TRN2", target_bir_lowering=False, debug=False,
                   num_devices=H)

    xTd = nc.dram_tensor("xTd", [128, NT * 4 * 128], bf16, kind="ExternalInput")
    Wd = nc.dram_tensor("Wd", [128, 4 * 128], bf16, kind="ExternalInput")
    attd = nc.dram_tensor("attd", [128, C], bf16, kind="ExternalInput")
    biasd = nc.dram_tensor("biasd", [128, C], f32, kind="ExternalInput")
    xresd = nc.dram_tensor("xresd", [NPAD, C], f32, kind="ExternalInput")
    srcd = nc.dram_tensor("srcd", [128, 8 * sumBS], i16, kind="ExternalInput")
    noded = nc.dram_tensor("noded", [128, 8 * NT], i16, kind="ExternalInput")
    maskd = nc.dram_tensor("maskd", [128, sumBS], f32, kind="ExternalInput")
    mask1d = nc.dram_tensor("mask1d", [128, sumBS], f32, kind="ExternalInput")
    tabled = nc.dram_tensor("tabled", [NPAD, 128], bf16)
    outd = nc.dram_tensor("out", [NPAD, C], f32, kind="ExternalOutput")

    with tile.TileContext(nc) as tc, ExitStack() as ctx:
        res = ctx.enter_context(tc.tile_pool(name="res", bufs=1))
        srci = res.tile([128, 8 * sumBS], i16, tag="srci")
        nc.sync.dma_start(srci[:], srcd.ap())
        nodei = res.tile([128, 8 * NT], i16, tag="nodei")
        nc.sync.dma_start(nodei[:], noded.ap())
        mask = res.tile([128, sumBS], f32, tag="mask")
        nc.sync.dma_start(mask[:], maskd.ap())
        mask1 = res.tile([128, sumBS], f32, tag="mask1")
        nc.sync.dma_start(mask1[:], mask1d.ap())
        att = res.tile([128, C], bf16, tag="att")
        nc.sync.dma_start(att[:], attd.ap())
        bias = res.tile([128, C], f32, tag="bias")
        nc.sync.dma_start(bias[:], biasd.ap())
        xres = res.tile([128, NT * C], f32, tag="xres")
        nc.sync.dma_start(
            xres[:].rearrange("p (t c) -> p t c", c=C),
            xresd.ap().rearrange("(t p) c -> p t c", p=128))
        aggst = res.tile([128, NT * C], f32, tag="aggst")
        denst = res.tile([128, NT], f32, tag="denst")

        # ---- Phase A: combined bf16 table [x_l | x_r] per node ----
        GT = 20  # tiles per xT streaming group
        with ExitStack() as actx:
            apool = actx.enter_context(tc.tile_pool(name="phA", bufs=2))
            wpool = actx.enter_context(tc.tile_pool(name="phA_w", bufs=1))
            psum = actx.enter_context(
                tc.tile_pool(name="phA_psum", bufs=4, space="PSUM"))
            w_sb = wpool.tile([128, 4 * 128], bf16, tag="w")
            nc.sync.dma_start(w_sb[:], Wd.ap())
            tstage = wpool.tile([128, NT * 128], bf16, tag="tstage")
            for g0 in range(0, NT, GT):
                g1 = min(g0 + GT, NT)
                xt = apool.tile([128, GT * 4 * 128], bf16, tag="xt")
                nc.sync.dma_start(xt[:, 0:(g1 - g0) * 4 * 128],
                                  xTd[:, g0 * 4 * 128:g1 * 4 * 128])
                for t in range(g0, g1):
                    ps = psum.tile([128, 128], f32, tag="ps")
                    for k in range(4):
                        lo = ((t - g0) * 4 + k) * 128
                        nc.tensor.matmul(ps[:], xt[:, lo:lo + 128],
                                         w_sb[:, k * 128:(k + 1) * 128],
                                         start=(k == 0), stop=(k == 3))
                    nc.scalar.copy(tstage[:, t * 128:(t + 1) * 128], ps[:])
            nc.sync.dma_start(
                tabled.ap().rearrange("(t p) c -> p t c", p=128),
                tstage[:].rearrange("p (t c) -> p t c", c=128))

        # ---- Phase B: batched edge phase ----
        with ExitStack() as bctx:
            gpool = bctx.enter_context(tc.tile_pool(name="phB_g", bufs=2))
            epool = bctx.enter_context(tc.tile_pool(name="phB_e", bufs=2))
            spool = bctx.enter_context(tc.tile_pool(name="phB_s", bufs=2))
            off = 0  # slot offset (in BS units)
            for (t0, B, S) in batches:
                BS = B * S
                # gathers: combined 256B rows
                Gt = gpool.tile([128, BS * 128], bf16, tag="G")
                nc.gpsimd.dma_gather(
                    Gt[:].rearrange("p (j c) -> p j c", c=128), tabled.ap(),
                    srci[:, 8 * off:8 * (off + BS)], 128 * BS, 128 * BS, 128,
                    single_packet=False)
                Rt = gpool.tile([128, B * 128], bf16, tag="R")
                nc.gpsimd.dma_gather(
                    Rt[:].rearrange("p (j c) -> p j c", c=128), tabled.ap(),
                    nodei[:, 8 * t0:8 * (t0 + B)], 128 * B, 128 * B, 128)

                G4 = Gt[:].rearrange("p (b s c) -> p b s c", s=S, c=128)
                Gl = G4[:, :, :, 0:C]            # x_l[src]
                Zr = G4[:, :, :, C:128]          # z workspace (r-half)
                Rr = Rt[:].rearrange("p (b c) -> p b c", c=128)[:, :, C:128]
                Rb = Rr.unsqueeze(2).broadcast_to([128, B, S, C])
                # z = x_l[src] + x_r[dst]  (bf16 2x)
                nc.vector.tensor_tensor(Zr, Gl, Rb, OP.add)
                # m = lrelu(z) on ACT (Prelu, slope 0.2), in place
                if os.environ.get("KERNEL_LRELU_DVE"):
                    nc.vector.scalar_tensor_tensor(Zr, Zr, NEG_SLOPE, Zr,
                                                   OP.mult, OP.max)
                else:
                    nc.scalar.activation(Zr, Zr, AF.Prelu, alpha=NEG_SLOPE)
                # v = m * att  (bf16 2x), in place
                Ab = att[:].unsqueeze(1).unsqueeze(1).broadcast_to(
                    [128, B, S, C])
                nc.vector.tensor_tensor(Zr, Zr, Ab, OP.mult)
                # alpha tree over c: bf16 folds 64->32->16->8
                w = C
                while w > 8:
                    h = w // 2
                    nc.vector.tensor_tensor(
                        G4[:, :, :, C:C + h], G4[:, :, :, C:C + h],
                        G4[:, :, :, C + h:C + 2 * h], OP.add)
                    w = h
                # f32 tail: 8->4 into alpha workspace, then 4->2->1
                aw = spool.tile([128, BS * 4], f32, tag="aw")
                a4 = aw[:].rearrange("p (j c) -> p j c", c=4)
                nc.vector.tensor_tensor(
                    a4, G4[:, :, :, C:C + 4].rearrange("p b s c -> p (b s) c"),
                    G4[:, :, :, C + 4:C + 8].rearrange("p b s c -> p (b s) c"),
                    OP.add)
                nc.vector.tensor_tensor(a4[:, :, 0:2], a4[:, :, 0:2],
                                        a4[:, :, 2:4], OP.add)
                aF = spool.tile([128, BS], f32, tag="aF")
                nc.vector.tensor_tensor(
                    aF[:], a4[:, :, 0:1].rearrange("p j c -> p (j c)"),
                    a4[:, :, 1:2].rearrange("p j c -> p (j c)"), OP.add)
                # shift by -segment_sum(alpha) (replicates the platform
                # reference, where segment_max actually computes segment_sum;
                # the 1e-16 denom clamp then has real effect)
                am0 = spool.tile([128, BS], f32, tag="am0")
                nc.vector.tensor_tensor(am0[:], aF[:],
                                        mask1[:, off:off + BS], OP.mult)
                sh = spool.tile([128, B], f32, tag="sh")
                nc.vector.tensor_reduce(
                    sh[:], am0[:].rearrange("p (b s) -> p b s", s=S),
                    AX.X, OP.add, negate=True)
                # mask pads to -1e30 (exp -> 0), apply shift, exp
                nc.vector.tensor_tensor(aF[:], aF[:],
                                        mask[:, off:off + BS], OP.add)
                aB = aF[:].rearrange("p (b s) -> p b s", s=S)
                shb = sh[:].unsqueeze(2).broadcast_to([128, B, S])
                nc.vector.tensor_tensor(aB, aB, shb, OP.add)
                eas = spool.tile([128, BS], f32, tag="eas")
                nc.scalar.activation(eas[:], aF[:], AF.Exp)
                # den per tile -> denstage
                nc.vector.tensor_reduce(
                    denst[:, t0:t0 + B],
                    eas[:].rearrange("p (b s) -> p b s", s=S), AX.X, OP.add)
                # expand ea to [B,S,C] bf16 on ACT
                ea = epool.tile([128, BS * C], bf16, tag="ea")
                ea3 = ea[:].rearrange("p (j c) -> p j c", c=C)
                easb = eas[:].unsqueeze(2).broadcast_to([128, BS, C])
                nc.scalar.activation(ea3, easb, AF.Copy)
                # wg = x_l[src] * ea  (bf16 2x), in place over ea
                nc.vector.tensor_tensor(
                    ea3, Gl.rearrange("p b s c -> p (b s) c"), ea3, OP.mult)
                # agg tree over s (per tile b), f32 tail into aggstage
                ea4 = ea[:].rearrange("p (b s c) -> p b s c", s=S, c=C)
                for (w, h) in _fold_widths(S):
                    lo = w - w // 2
                    if w == 2:
                        nc.vector.tensor_tensor(
                            aggst[:].rearrange("p (t c) -> p t c", c=C)
                            [:, t0:t0 + B, :],
                            ea4[:, :, 0, :], ea4[:, :, 1, :], OP.add)
                    else:
                        nc.vector.tensor_tensor(
                            ea4[:, :, 0:w - lo, :], ea4[:, :, 0:w - lo, :],
                            ea4[:, :, lo:w, :], OP.add)
                off += BS

        # ---- Finale: a2 = agg/den + bias; out = elu(a2) + xres ----
        with ExitStack() as fctx:
            fpool = fctx.enter_context(tc.tile_pool(name="fin", bufs=1))
            rd = fpool.tile([128, NT], f32, tag="rd")
            nc.vector.tensor_scalar_max(rd[:], denst[:], 1e-16)
            nc.vector.reciprocal(rd[:], rd[:])
            ag3 = aggst[:].rearrange("p (t c) -> p t c", c=C)
            rdb = rd[:].unsqueeze(2).broadcast_to([128, NT, C])
            nc.vector.tensor_tensor(ag3, ag3, rdb, OP.mult)
            bb = bias[:].unsqueeze(1).broadcast_to([128, NT, C])
            nc.vector.tensor_tensor(ag3, ag3, bb, OP.add)
            tmp = fpool.tile([128, NT * C], f32, tag="tmp")
            nc.vector.tensor_scalar_min(tmp[:], aggst[:], 0.0)
            nc.scalar.activation(tmp[:], tmp[:], AF.Exp)
            nc.vector.scalar_tensor_tensor(aggst[:], aggst[:], 0.0, tmp[:],
                                           OP.max, OP.add)
            nc.vector.scalar_tensor_tensor(aggst[:], aggst[:], -1.0, xres[:],
                                           OP.add, OP.add)
            nc.sync.dma_start(
                outd.ap().rearrange("(t p) c -> p t c", p=128),
                aggst[:].rearrange("p (t c) -> p t c", c=C))

    nc.compile()
    return nc


# ----------------------------------------------------------------------------
# Per-core input assembly + driver
# ----------------------------------------------------------------------------

def _make_in_maps(x, W_l, W_r, att, bias, prep):
    xp = np.zeros((NPAD, D), BF16)
    xp[:N] = x.astype(BF16)
    # xTd[c, t, k, j] = x[t*128+j, k*128+c]
    xTd = np.ascontiguousarray(
        xp.reshape(NT, 128, 4, 128).transpose(3, 0, 2, 1).reshape(128, -1))
    xres_all = x[prep["perm_clip"]]  # [NPAD, D] f32

    in_maps = []
    for h in range(H):
        cs = slice(h * C, (h + 1) * C)
        Wc = np.empty((128, 4 * 128), BF16)
        for k in range(4):
            Wc[:, k * 128:k * 128 + C] = W_l[k * 128:(k + 1) * 128, cs]
            Wc[:, k * 128 + C:(k + 1) * 128] = W_r[k * 128:(k + 1) * 128, cs]
        in_maps.append({
            "xTd": xTd,
            "Wd": Wc,
            "attd": np.ascontiguousarray(np.tile(att[h].astype(BF16), (128, 1))),
            "biasd": np.ascontiguousarray(
                np.tile(bias[cs], (128, 1))).astype(np.float32),
            "xresd": np.ascontiguousarray(xres_all[:, cs]).astype(np.float32),
            "srcd": prep["srcidx"],
            "noded": prep["nodeidx"],
            "maskd": prep["mask"],
            "mask1d": prep["mask01"],
        })
    return in_maps


def _get_program(batches, sumBS):
    key = tuple(batches)
    if key not in _CACHE:
        _CACHE[key] = _build(list(batches), sumBS)
    return _CACHE[key]


_LAST = {}


def kernel(**inputs):
    x = np.asarray(inputs["x"], np.float32)
    edge_index = np.asarray(inputs["edge_index"])
    W_l = np.asarray(inputs["W_l"], np.float32)
    W_r = np.asarray(inputs["W_r"], np.float32)
    att = np.asarray(inputs["att"], np.float32)
    bias = np.asarray(inputs["bias"], np.float32)

    prep = _prep(edge_index)
    sumBS = sum(b * s for (_, b, s) in prep["batches"])
    nc = _get_program(prep["batches"], sumBS)
    in_maps = _make_in_maps(x, W_l, W_r, att, bias, prep)

    from concourse.bass_utils import run_bass_kernel_spmd
    bkr = run_bass_kernel_spmd(nc, in_maps, core_ids=list(range(H)))

    out = np.empty((N, D), np.float32)
    for h in range(H):
        out[prep["perm"], h * C:(h + 1) * C] = bkr.results[h]["out"][:N]

    _LAST["nc"] = nc
    _LAST["in_maps"] = in_maps
    _LAST["prep"] = prep
    return out


def _time_pjrt(nc, in_maps, ks=(1, 17), reps=5):
    """Time NEFF execution through PJRT with device-resident inputs."""
    import time
    import jax
    import numpy as _np
    from jax.sharding import Mesh, PartitionSpec, NamedSharding
    from jax.experimental.shard_map import shard_map
    from concourse import mybir
    from concourse.bass2jax import (_bass_exec_p, install_neuronx_cc_hook,
                                    partition_id_tensor)

    install_neuronx_cc_hook()
    n_cores = len(in_maps)
    partition_name = nc.partition_id_tensor.name if nc.partition_id_tensor else None
    in_names, out_names, out_avals, zero_outs = [], [], [], []
    for alloc in nc.m.functions[0].allocations:
        if not isinstance(alloc, mybir.MemoryLocationSet):
            continue
        name = alloc.memorylocations[0].name
        if alloc.kind == "ExternalInput":
            if name != partition_name:
                in_names.append(name)
        elif alloc.kind == "ExternalOutput":
            out_names.append(name)
            shape = tuple(alloc.tensor_shape)
            dtype = mybir.dt.np(alloc.dtype)
            out_avals.append(jax.core.ShapedArray(shape, dtype))
            zero_outs.append(_np.zeros(shape, dtype))
    n_params = len(in_names)
    full_in_names = in_names + out_names + ([partition_name] if partition_name else [])
    donate = tuple(range(n_params, n_params + len(out_names)))

    def _body(*args):
        operands = list(args)
        if partition_name is not None:
            operands.append(partition_id_tensor())
        return tuple(_bass_exec_p.bind(
            *operands, out_avals=tuple(out_avals), in_names=tuple(full_in_names),
            out_names=tuple(out_names), lowering_input_output_aliases=(),
            sim_require_finite=True, sim_require_nnan=True, nc=nc))

    devices = jax.devices()[:n_cores]
    mesh = Mesh(_np.asarray(devices), ("core",))
    spec = NamedSharding(mesh, PartitionSpec("core"))
    in_specs = (PartitionSpec("core"),) * (n_params + len(out_names))
    out_specs = (PartitionSpec("core"),) * len(out_names)
    fn = jax.jit(shard_map(_body, mesh=mesh, in_specs=in_specs,
                           out_specs=out_specs, check_rep=False),
                 donate_argnums=donate, keep_unused=True)
    concat_in = [jax.device_put(
        _np.concatenate([_np.asarray(in_maps[c][nm]) for c in range(n_cores)], axis=0),
        spec) for nm in in_names]

    def timed_chain(k):
        zero_sets = []
        for _ in range(k):
            zs = [jax.device_put(
                _np.zeros((n_cores * z.shape[0], *z.shape[1:]), z.dtype), spec)
                for z in zero_outs]
            for a in zs:
                a.block_until_ready()
            zero_sets.append(zs)
        t0 = time.perf_counter()
        outs = None
        for zs in zero_sets:
            outs = fn(*concat_in, *zs)
        for o in outs:
            o.block_until_ready()
        return time.perf_counter() - t0

    timed_chain(1)  # warm
    times = {}
    for k in ks:
        times[k] = min(timed_chain(k) for _ in range(reps))
    return times


def profile_exec_ns():
    """Slope-based timing through PJRT: marginal cost per call
    = (t_K - t_1) / (K - 1)."""
    assert "nc" in _LAST, "call kernel() first"
    ks = (1, 17)
    tk = _time_pjrt(_LAST["nc"], _LAST["in_maps"], ks=ks)
    ns = int((tk[ks[1]] - tk[ks[0]]) / (ks[1] - ks[0]) * 1e9)
    return ns, {"kernel_chain_s": tk}


# revision 27
# speedup vs baseline: 2.1280x; 1.9649x over previous
"""GATv2Conv (PyG semantics) on 8 Trainium2 NeuronCores — v3.

Sharding: destination-node sharding (graph-parallel, per the sharding hint).
Degree-sorted dst tiles of 128 nodes are dealt round-robin to the 8 cores
(core c takes sorted tile 8k+c of group k, padded to the group-max slot
count S_k so the SPMD program is identical across cores). Each core
processes ALL 8 heads for its ~10 tiles:

  Phase A: full x_l table [NPAD, 512] bf16 in HBM (bf16 matmuls on PE),
           plus x_r rows for the core's own dst nodes, kept in SBUF.
  Phase B: per (tile, head-group) half-pipeline: one 512B-row dma_gather
           brings x_l[src] for 4 heads (the per-row descriptor rate is the
           HW bottleneck — 512B rows are the measured sweet spot), then
           z-add / Prelu / att-mult / alpha fold-tree / segment-sum shift /
           exp / weighted fold-tree aggregation, bf16 on DVE+ACT.
  Finale: ELU + residual over all tiles at once, one output DMA.

kernel(**inputs) takes full unsharded inputs, returns full [10000, 512] f32.
"""

import os
import sys
from contextlib import ExitStack

for _p in ("/opt/trn_rl_repo",):
    if _p not in sys.path:
        sys.path.insert(0, _p)

import numpy as np
import ml_dtypes

N, E, D, H = 10000, 160000, 512, 8
C = D // H  # 64
NT = (N + 127) // 128  # 79 node tiles
NTC = (NT + 7) // 8  # 10 tiles per core (last may be empty)
NPAD = NT * 128  # 10112
HG = 2           # head groups per tile
HW_ = D // HG    # 256 channels per head group
NEG_SLOPE = 0.2
F16 = np.float16

_CACHE = {}


def _wrap_idx(idx_flat):
    """SWDGE index layout: logical j at [j % 16, j // 16] of a [16, L] block,
    replicated across the 8 Q7 core stripes -> [128, L] int16."""
    n = len(idx_flat)
    lw = (n + 15) // 16
    buf = np.zeros((16, lw), np.int16)
    j = np.arange(n)
    buf[j % 16, j // 16] = idx_flat.astype(np.int16)
    return np.tile(buf, (8, 1))


def _prep(edge_index):
    ei = np.asarray(edge_index).astype(np.int64)
    src = np.concatenate([ei[0], np.arange(N, dtype=np.int64)])
    dst = np.concatenate([ei[1], np.arange(N, dtype=np.int64)])
    deg = np.bincount(dst, minlength=N)  # >= 1 (self loops)
    order = np.argsort(dst, kind="stable")
    src_sorted = src[order]
    starts = np.zeros(N + 1, np.int64)
    starts[1:] = np.cumsum(deg)
    perm = np.argsort(-deg, kind="stable")  # descending degree
    perm_full = np.concatenate([perm, np.full(NPAD - N, -1, np.int64)])

    # group k = sorted tiles [8k, 8k+8); S_k = group max slot count (even)
    S_list = []
    for k in range(NTC):
        t0 = 8 * k
        nodes = perm_full[t0 * 128:min(NT, t0 + 8) * 128]
        degs = np.where(nodes >= 0, deg[np.clip(nodes, 0, N - 1)], 1)
        S = int(degs.max())
        S += S & 1
        S_list.append(max(S, 2))

    # per-core slot tables
    srcidx, masks, mask01s, tiles_of = [], [], [], []
    for c in range(H):
        blocks, mblocks = [], []
        tids = []
        for k, S in enumerate(S_list):
            t = 8 * k + c
            tids.append(t)
            blk = np.zeros((S, 128), np.int64)
            msk = np.full((128, S), -1e30, np.float32)
            if t < NT:
                nodes = perm_full[t * 128:(t + 1) * 128]
                for p in range(128):
                    nd = nodes[p]
                    if nd < 0:
                        continue
                    d_ = int(deg[nd])
                    blk[:d_, p] = src_sorted[starts[nd]:starts[nd] + d_]
                    msk[p, :d_] = 0.0
            blocks.append(blk.reshape(-1))  # logical j = s*128 + p
            mblocks.append(msk)
        srcidx.append(_wrap_idx(np.concatenate(blocks)))
        m = np.concatenate(mblocks, axis=1)  # [128, sumS]
        masks.append(m)
        mask01s.append((m == 0.0).astype(np.float32))
        tiles_of.append(tids)
    perm_clip = np.where(perm_full >= 0, perm_full, 0)
    return dict(S_list=S_list, srcidx=srcidx, masks=masks, mask01s=mask01s,
                tiles_of=tiles_of, perm=perm, perm_full=perm_full,
                perm_clip=perm_clip)


def _fold_widths(S):
    out = []
    w = S
    while w > 1:
        h = w - w // 2
        out.append((w, h))
        w = h
    return out


def _build(S_list, stage=9):
    import concourse.bacc as bacc
    import concourse.tile as tile
    from concourse import mybir

    f32 = mybir.dt.float32
    f16 = mybir.dt.float16
    i16 = mybir.dt.int16
    AF = mybir.ActivationFunctionType
    OP = mybir.AluOpType
    AX = mybir.AxisListType

    sumS = sum(S_list)
    nc = bacc.Bacc("TRN2", target_bir_lowering=False, debug=False,
                   num_devices=H)

    xTd = nc.dram_tensor("xTd", [128, NT * 4 * 128], f16, kind="ExternalInput")
    xrd = nc.dram_tensor("xrd", [128, NTC * 4 * 128], f16, kind="ExternalInput")
    Wld = nc.dram_tensor("Wld", [128, 4 * 512], f16, kind="ExternalInput")
    Wrd = nc.dram_tensor("Wrd", [128, 4 * 512], f16, kind="ExternalInput")
    attd = nc.dram_tensor("attd", [128, 512], f16, kind="ExternalInput")
    biasd = nc.dram_tensor("biasd", [128, 512], f32, kind="ExternalInput")
    xresd = nc.dram_tensor("xresd", [NTC * 128, 512], f32, kind="ExternalInput")
    srcd = nc.dram_tensor("srcd", [128, 8 * sumS], i16, kind="ExternalInput")
    maskd = nc.dram_tensor("maskd", [128, sumS], f32, kind="ExternalInput")
    mask1d = nc.dram_tensor("mask1d", [128, sumS], f32, kind="ExternalInput")
    tabled0 = nc.dram_tensor("tabled0", [NPAD, 256], f16)
    tabled1 = nc.dram_tensor("tabled1", [NPAD, 256], f16)
    outd = nc.dram_tensor("out", [NTC * 128, 512], f32, kind="ExternalOutput")

    with tile.TileContext(nc) as tc, ExitStack() as ctx:
        res = ctx.enter_context(tc.tile_pool(name="res", bufs=1))
        srci = res.tile([128, 8 * sumS], i16, tag="srci")
        nc.sync.dma_start(srci[:], srcd.ap())
        mask = res.tile([128, sumS], f32, tag="mask")
        nc.sync.dma_start(mask[:], maskd.ap())
        mask1 = res.tile([128, sumS], f32, tag="mask1")
        nc.sync.dma_start(mask1[:], mask1d.ap())
        att = res.tile([128, 512], f16, tag="att")
        nc.sync.dma_start(att[:], attd.ap())
        bias = res.tile([128, 512], f32, tag="bias")
        nc.sync.dma_start(bias[:], biasd.ap())
        xres = res.tile([128, NTC * 512], f32, tag="xres")
        nc.sync.dma_start(
            xres[:].rearrange("p (t c) -> p t c", c=512),
            xresd.ap().rearrange("(t p) c -> p t c", p=128))
        aggst = res.tile([128, NTC * 512], f32, tag="aggst")
        rsb = res.tile([128, NTC * 512], f16, tag="rsb")  # x_r own rows
        if stage < 9:
            nc.gpsimd.memset(aggst[:], 0.0)

        # ---- Phase A ----
        GT = 20
        if stage >= 1:
            with ExitStack() as actx:
                apool = actx.enter_context(tc.tile_pool(name="phA", bufs=2))
                wpool = actx.enter_context(tc.tile_pool(name="phA_w", bufs=1))
                psum = actx.enter_context(
                    tc.tile_pool(name="phA_psum", bufs=2, space="PSUM"))
                stg = actx.enter_context(tc.tile_pool(name="phA_st", bufs=2))
                wl = wpool.tile([128, 4 * 512], f16, tag="wl")
                nc.sync.dma_start(wl[:], Wld.ap())
                wr = wpool.tile([128, 4 * 512], f16, tag="wr")
                nc.sync.dma_start(wr[:], Wrd.ap())
                # x_r rows for own tiles -> SBUF (stay resident)
                xrt = wpool.tile([128, NTC * 4 * 128], f16, tag="xrt")
                nc.sync.dma_start(xrt[:], xrd.ap())
                for k in range(NTC):
                    ps = psum.tile([128, 512], f32, tag="psr")
                    for q in range(4):
                        lo = (k * 4 + q) * 128
                        nc.tensor.matmul(ps[:], xrt[:, lo:lo + 128],
                                         wr[:, q * 512:(q + 1) * 512],
                                         start=(q == 0), stop=(q == 3))
                    nc.scalar.copy(rsb[:, k * 512:(k + 1) * 512], ps[:])
                # full x_l table -> HBM, streamed by tile group
                for g0 in range(0, NT, GT):
                    g1 = min(g0 + GT, NT)
                    xt = apool.tile([128, GT * 4 * 128], f16, tag="xt")
                    nc.sync.dma_start(xt[:, 0:(g1 - g0) * 4 * 128],
                                      xTd[:, g0 * 4 * 128:g1 * 4 * 128])
                    ts0 = stg.tile([128, GT * 256], f16, tag="ts0")
                    ts1 = stg.tile([128, GT * 256], f16, tag="ts1")
                    for t in range(g0, g1):
                        ps = psum.tile([128, 512], f32, tag="psl")
                        for q in range(4):
                            lo = ((t - g0) * 4 + q) * 128
                            nc.tensor.matmul(ps[:], xt[:, lo:lo + 128],
                                             wl[:, q * 512:(q + 1) * 512],
                                             start=(q == 0), stop=(q == 3))
                        nc.scalar.copy(
                            ts0[:, (t - g0) * 256:(t - g0 + 1) * 256],
                            ps[:, 0:256])
                        nc.scalar.copy(
                            ts1[:, (t - g0) * 256:(t - g0 + 1) * 256],
                            ps[:, 256:512])
                    for hgx, (tsx, tabx) in enumerate(
                            [(ts0, tabled0), (ts1, tabled1)]):
                        nc.sync.dma_start(
                            tabx.ap()[g0 * 128:g1 * 128, :]
                            .rearrange("(t p) c -> p t c", p=128),
                            tsx[:, 0:(g1 - g0) * 256]
                            .rearrange("p (t c) -> p t c", c=256))

        # ---- Phase B: per (tile, head-group) half-pipelines ----
        with ExitStack() as bctx:
            gpool = bctx.enter_context(tc.tile_pool(name="phB_g", bufs=2))
            zpool = bctx.enter_context(tc.tile_pool(name="phB_z", bufs=2))
            epool = bctx.enter_context(tc.tile_pool(name="phB_e", bufs=2))
            spool = bctx.enter_context(tc.tile_pool(name="phB_s", bufs=2))
            off = 0
            for k, S in enumerate(S_list):
                if stage < 2:
                    break
                for hg in range(HG):
                    co = hg * HW_  # channel offset of this head group
                    # gather x_l[src] 512B slices (4 heads)
                    Gt = gpool.tile([128, S * HW_], f16, tag="G")
                    nc.gpsimd.dma_gather(
                        Gt[:].rearrange("p (j c) -> p j c", c=HW_),
                        (tabled0 if hg == 0 else tabled1).ap(),
                        srci[:, 8 * off:8 * (off + S)], 128 * S, 128 * S,
                        HW_, single_packet=False)
                    if stage < 3:
                        continue
                    G3 = Gt[:].rearrange("p (s c) -> p s c", c=HW_)
                    # z = x_l[src] + x_r[dst]
                    z = zpool.tile([128, S * HW_], f16, tag="z")
                    z3 = z[:].rearrange("p (s c) -> p s c", c=HW_)
                    Rb = rsb[:, k * 512 + co:k * 512 + co + HW_] \
                        .unsqueeze(1).broadcast_to([128, S, HW_])
                    nc.vector.tensor_tensor(z3, G3, Rb, OP.add)
                    # m = lrelu(z) in place
                    if os.environ.get("KERNEL_LRELU_DVE"):
                        nc.vector.scalar_tensor_tensor(
                            z[:], z[:], NEG_SLOPE, z[:], OP.mult, OP.max)
                    else:
                        nc.scalar.activation(z[:], z[:], AF.Prelu,
                                             alpha=NEG_SLOPE)
                    # v = m * att in place
                    Ab = att[:, co:co + HW_].unsqueeze(1) \
                        .broadcast_to([128, S, HW_])
                    nc.vector.tensor_tensor(z3, z3, Ab, OP.mult)
                    # alpha tree over c within each head: 64->32->16->8 bf16
                    z4 = z[:].rearrange("p (s h c) -> p s h c", h=4, c=C)
                    w = C
                    while w > 8:
                        hw2 = w // 2
                        nc.vector.tensor_tensor(
                            z4[:, :, :, 0:hw2], z4[:, :, :, 0:hw2],
                            z4[:, :, :, hw2:w], OP.add)
                        w = hw2
                    aw = spool.tile([128, S * 4 * 4], f32, tag="aw")
                    a4 = aw[:].rearrange("p (j c) -> p j c", c=4)
                    nc.vector.tensor_tensor(
                        a4, z4[:, :, :, 0:4].rearrange("p s h c -> p (s h) c"),
                        z4[:, :, :, 4:8].rearrange("p s h c -> p (s h) c"),
                        OP.add)
                    nc.vector.tensor_tensor(a4[:, :, 0:2], a4[:, :, 0:2],
                                            a4[:, :, 2:4], OP.add)
                    aF = spool.tile([128, S * 4], f32, tag="aF")
                    nc.vector.tensor_tensor(
                        aF[:], a4[:, :, 0:1].rearrange("p j c -> p (j c)"),
                        a4[:, :, 1:2].rearrange("p j c -> p (j c)"), OP.add)
                    if stage < 4:
                        continue
                    # shift = -segment_sum(alpha) (platform reference
                    # semantics: its segment_max computes segment_sum, and
                    # the 1e-16 denom clamp then has real effect)
                    aFh = aF[:].rearrange("p (s h) -> p s h", h=4)
                    am0 = spool.tile([128, S * 4], f32, tag="am0")
                    m1b = mask1[:, off:off + S].unsqueeze(2) \
                        .broadcast_to([128, S, 4])
                    nc.vector.tensor_tensor(
                        am0[:].rearrange("p (s h) -> p s h", h=4), aFh,
                        m1b, OP.mult)
                    sh = spool.tile([128, 4], f32, tag="sh")
                    nc.vector.tensor_reduce(
                        sh[:], am0[:].rearrange("p (s h) -> p h s", h=4),
                        AX.X, OP.add, negate=True)
                    mkb = mask[:, off:off + S].unsqueeze(2) \
                        .broadcast_to([128, S, 4])
                    nc.vector.tensor_tensor(aFh, aFh, mkb, OP.add)
                    shb = sh[:].unsqueeze(1).broadcast_to([128, S, 4])
                    nc.vector.tensor_tensor(aFh, aFh, shb, OP.add)
                    eas = spool.tile([128, S * 4], f32, tag="eas")
                    nc.scalar.activation(eas[:], aF[:], AF.Exp)
                    den = spool.tile([128, 4], f32, tag="den")
                    nc.vector.tensor_reduce(
                        den[:], eas[:].rearrange("p (s h) -> p h s", h=4),
                        AX.X, OP.add)
                    rdn = spool.tile([128, 4], f32, tag="rdn")
                    nc.vector.tensor_scalar_max(rdn[:], den[:], 1e-16)
                    nc.vector.reciprocal(rdn[:], rdn[:])
                    # w = ea / max(den, 1e-16)  (<= 1, fp16-safe)
                    rdb = rdn[:].unsqueeze(1).broadcast_to([128, S, 4])
                    nc.vector.tensor_tensor(
                        eas[:].rearrange("p (s h) -> p s h", h=4),
                        eas[:].rearrange("p (s h) -> p s h", h=4), rdb,
                        OP.mult)
                    # expand w over c on ACT
                    ea = epool.tile([128, S * HW_], f16, tag="ea")
                    ea3 = ea[:].rearrange("p (j c) -> p j c", c=C)
                    easb = eas[:].unsqueeze(2).broadcast_to([128, S * 4, C])
                    nc.scalar.activation(ea3, easb, AF.Copy)
                    if stage < 5:
                        continue
                    # wg = x_l[src] * ea, in place over ea
                    nc.vector.tensor_tensor(ea[:], Gt[:], ea[:], OP.mult)
                    # agg tree over s, f32 tail into aggstage
                    ea4 = ea[:].rearrange("p (s c) -> p s c", c=HW_)
                    for (w, h2) in _fold_widths(S):
                        lo = w - w // 2
                        if w == 2:
                            nc.vector.tensor_tensor(
                                aggst[:, k * 512 + co:k * 512 + co + HW_],
                                ea4[:, 0, :], ea4[:, 1, :], OP.add)
                        else:
                            nc.vector.tensor_tensor(
                                ea4[:, 0:w - lo, :], ea4[:, 0:w - lo, :],
                                ea4[:, lo:w, :], OP.add)
                off += S

        # ---- Finale ----
        with ExitStack() as fctx:
            fpool = fctx.enter_context(tc.tile_pool(name="fin", bufs=1))
            bb = bias[:].unsqueeze(1).broadcast_to([128, NTC, 512])
            ag2 = aggst[:].rearrange("p (t c) -> p t c", c=512)
            nc.vector.tensor_tensor(ag2, ag2, bb, OP.add)
            tmp = fpool.tile([128, NTC * 512], f32, tag="tmp")
            nc.vector.tensor_scalar_min(tmp[:], aggst[:], 0.0)
            nc.scalar.activation(tmp[:], tmp[:], AF.Exp)
            nc.vector.scalar_tensor_tensor(aggst[:], aggst[:], 0.0, tmp[:],
                                           OP.max, OP.add)
            nc.vector.scalar_tensor_tensor(aggst[:], aggst[:], -1.0, xres[:],
                                           OP.add, OP.add)
            nc.sync.dma_start(
                outd.ap().rearrange("(t p) c -> p t c", p=128),
                aggst[:].rearrange("p (t c) -> p t c", c=512))

    nc.compile()
    return nc


# ----------------------------------------------------------------------------
# Per-core input assembly + driver
# ----------------------------------------------------------------------------

def _make_in_maps(x, W_l, W_r, att, bias, prep):
    xp = np.zeros((NPAD, D), F16)
    xp[:N] = x.astype(F16)
    # xTd[c, t, k, j] = x[t*128+j, k*128+c]
    xT4 = xp.reshape(NT, 128, 4, 128).transpose(3, 0, 2, 1)
    xTd = np.ascontiguousarray(xT4.reshape(128, -1))
    # permuted-node view for per-core x_r rows (tiles hold permuted nodes)
    xT4p = xp[prep["perm_clip"]].reshape(NT, 128, 4, 128).transpose(3, 0, 2, 1)

    def wchunks(W):
        return np.ascontiguousarray(
            W.reshape(4, 128, 512).transpose(1, 0, 2)
            .reshape(128, 4 * 512)).astype(F16)

    Wlc = wchunks(W_l)
    Wrc = wchunks(W_r)
    attf = np.ascontiguousarray(np.tile(att.reshape(-1).astype(F16), (128, 1)))
    biasf = np.ascontiguousarray(np.tile(bias, (128, 1))).astype(np.float32)
    xres_all = x[prep["perm_clip"]]  # [NPAD, D] f32

    in_maps = []
    for c in range(H):
        tids = prep["tiles_of"][c]
        # xrd: x columns for own tiles (canonical layout, per-core data)
        xr = np.zeros((128, NTC, 4, 128), F16)
        xre = np.zeros((NTC * 128, D), np.float32)
        for k, t in enumerate(tids):
            if t < NT:
                xr[:, k] = xT4p[:, t]
                xre[k * 128:(k + 1) * 128] = xres_all[t * 128:(t + 1) * 128]
        in_maps.append({
            "xTd": xTd,
            "xrd": np.ascontiguousarray(xr.reshape(128, -1)),
            "Wld": Wlc,
            "Wrd": Wrc,
            "attd": attf,
            "biasd": biasf,
            "xresd": xre,
            "srcd": prep["srcidx"][c],
            "maskd": prep["masks"][c],
            "mask1d": prep["mask01s"][c],
        })
    return in_maps


def _get_program(S_list):
    key = tuple(S_list)
    if key not in _CACHE:
        _CACHE[key] = _build(list(S_list))
    return _CACHE[key]


_LAST = {}


def kernel(**inputs):
    x = np.asarray(inputs["x"], np.float32)
    edge_index = np.asarray(inputs["edge_index"])
    W_l = np.asarray(inputs["W_l"], np.float32)
    W_r = np.asarray(inputs["W_r"], np.float32)
    att = np.asarray(inputs["att"], np.float32)
    bias = np.asarray(inputs["bias"], np.float32)

    prep = _prep(edge_index)
    nc = _get_program(prep["S_list"])
    in_maps = _make_in_maps(x, W_l, W_r, att, bias, prep)

    from concourse.bass_utils import run_bass_kernel_spmd
    bkr = run_bass_kernel_spmd(nc, in_maps, core_ids=list(range(H)))

    out = np.empty((N, D), np.float32)
    pf = prep["perm_full"]
    for c in range(H):
        oc = bkr.results[c]["out"]  # [NTC*128, 512]
        for k, t in enumerate(prep["tiles_of"][c]):
            if t >= NT:
                continue
            nodes = pf[t * 128:(t + 1) * 128]
            sel = nodes >= 0
            out[nodes[sel]] = oc[k * 128:(k + 1) * 128][sel]

    _LAST["nc"] = nc
    _LAST["in_maps"] = in_maps
    _LAST["prep"] = prep
    return out


def _time_pjrt(nc, in_maps, ks=(1, 17), reps=5):
    """Time NEFF execution through PJRT with device-resident inputs."""
    import time
    import jax
    import numpy as _np
    from jax.sharding import Mesh, PartitionSpec, NamedSharding
    from jax.experimental.shard_map import shard_map
    from concourse import mybir
    from concourse.bass2jax import (_bass_exec_p, install_neuronx_cc_hook,
                                    partition_id_tensor)

    install_neuronx_cc_hook()
    n_cores = len(in_maps)
    partition_name = nc.partition_id_tensor.name if nc.partition_id_tensor else None
    in_names, out_names, out_avals, zero_outs = [], [], [], []
    for alloc in nc.m.functions[0].allocations:
        if not isinstance(alloc, mybir.MemoryLocationSet):
            continue
        name = alloc.memorylocations[0].name
        if alloc.kind == "ExternalInput":
            if name != partition_name:
                in_names.append(name)
        elif alloc.kind == "ExternalOutput":
            out_names.append(name)
            shape = tuple(alloc.tensor_shape)
            dtype = mybir.dt.np(alloc.dtype)
            out_avals.append(jax.core.ShapedArray(shape, dtype))
            zero_outs.append(_np.zeros(shape, dtype))
    n_params = len(in_names)
    full_in_names = in_names + out_names + ([partition_name] if partition_name else [])
    donate = tuple(range(n_params, n_params + len(out_names)))

    def _body(*args):
        operands = list(args)
        if partition_name is not None:
            operands.append(partition_id_tensor())
        return tuple(_bass_exec_p.bind(
            *operands, out_avals=tuple(out_avals), in_names=tuple(full_in_names),
            out_names=tuple(out_names), lowering_input_output_aliases=(),
            sim_require_finite=True, sim_require_nnan=True, nc=nc))

    devices = jax.devices()[:n_cores]
    mesh = Mesh(_np.asarray(devices), ("core",))
    spec = NamedSharding(mesh, PartitionSpec("core"))
    in_specs = (PartitionSpec("core"),) * (n_params + len(out_names))
    out_specs = (PartitionSpec("core"),) * len(out_names)
    fn = jax.jit(shard_map(_body, mesh=mesh, in_specs=in_specs,
                           out_specs=out_specs, check_rep=False),
                 donate_argnums=donate, keep_unused=True)
    concat_in = [jax.device_put(
        _np.concatenate([_np.asarray(in_maps[c][nm]) for c in range(n_cores)], axis=0),
        spec) for nm in in_names]

    def timed_chain(k):
        zero_sets = []
        for _ in range(k):
            zs = [jax.device_put(
                _np.zeros((n_cores * z.shape[0], *z.shape[1:]), z.dtype), spec)
                for z in zero_outs]
            for a in zs:
                a.block_until_ready()
            zero_sets.append(zs)
        t0 = time.perf_counter()
        outs = None
        for zs in zero_sets:
            outs = fn(*concat_in, *zs)
        for o in outs:
            o.block_until_ready()
        return time.perf_counter() - t0

    timed_chain(1)  # warm
    times = {}
    for k in ks:
        times[k] = min(timed_chain(k) for _ in range(reps))
    return times


def profile_exec_ns():
    """Slope-based timing through PJRT: marginal cost per call
    = (t_K - t_1) / (K - 1)."""
    assert "nc" in _LAST, "call kernel() first"
    ks = (1, 17)
    tk = _time_pjrt(_LAST["nc"], _LAST["in_maps"], ks=ks)
    ns = int((tk[ks[1]] - tk[ks[0]]) / (ks[1] - ks[0]) * 1e9)
    return ns, {"kernel_chain_s": tk}
